# revision 7
# baseline (speedup 1.0000x reference)
"""Trainium2 Bass kernel for nn_MultiHeadAttention_2250562863251.

Key algebraic insight: the reference einsum 'mbhi,nbhj->mnbh' contracts i and j
independently, so scores[m,n,b,h] = (sum_i q[m,b,h,i]) * (sum_j k[n,b,h,j]) —
a rank-1 outer product of per-head row-sums. Full Q/K projections are never
needed; only queries @ (per-head-summed Wq) [E,H], computed on host (tiny).

Sharding: 8 cores = 2 (batch) x 4 (head-groups of 4 heads).

Device per core (batch bi, heads hg*4..hg*4+3):
  - v-proj:  v = values_b @ WvL.T   (PE, fp32r, via host-transposed valuesT)
  - scores:  scT[n,m] = qs_bcast[m]*c'_n + beta_bcast[m]   (DVE stt, fp32)
             + tri additive causal mask on diagonal blocks
  - exp:     eT = exp(scT + d_n)    (ACT, per-partition bias, bf16 out)
    where beta_m = -(qs_m * suffix-extreme(c)) = -rowmax (host), so the
    softmax max-subtraction is folded into the score build, and d_n = -1000
    padding mask folds into the ACT bias.
  - attn:    pooled[m, 65] accumulates eT.T @ [v_h | ones] over n-chunks
             (bf16 matmul; ones column yields the softmax denominator)
  - divide:  pooled[:, :64] * recip(pooled[:, 64])  per head
  - out-proj: outT[o, m] = WoLT.T @ pooledT (PE transpose + fp32r matmul)
Host assembles: out[m,b,:] = sum_hg outT.T + bo + bv @ Wo.T, with exact
recompute of (rare/absent) degenerate rows where rowmax <= -990.
"""
import sys

for _p in ("/opt/trn_rl_repo", "/root/.axon_site/_ro/trn_rl_repo"):
    if _p not in sys.path:
        sys.path.append(_p)

import numpy as np
import ml_dtypes

import concourse.bass as bass
import concourse.mybir as mybir
import concourse.tile as tile
from concourse import bacc
from concourse.bass_utils import run_bass_kernel_spmd
from concourse.masks import make_identity

# Problem shapes (hardcoded per contract)
M = 2048   # query positions
N = 2048   # key positions
B = 2
E = 1024
H = 16
DH = 64        # head dim
HL = 4         # heads per core
KL = HL * DH   # 256 local pooled dims
NEG = -1000.0
P = 128
NK = N // P    # 16 n-chunks
T = 4          # m-tiles of 512
MT = 512
NCORES = 8

f32 = mybir.dt.float32
f32r = mybir.dt.float32r
bf16 = mybir.dt.bfloat16

_CACHE = {}


def _build_program():
    if "nc" in _CACHE:
        return _CACHE["nc"]
    nc = bacc.Bacc("TRN2", target_bir_lowering=False, debug=False,
                   num_devices=NCORES)

    vt_d = nc.declare_dram_parameter("vt", [E, N], f32, isOutput=False)
    wvlt_d = nc.declare_dram_parameter("wvlt", [E, KL], f32, isOutput=False)
    wolt_d = nc.declare_dram_parameter("wolt", [KL, E], f32, isOutput=False)
    qsl_d = nc.declare_dram_parameter("qsl", [HL, M], f32, isOutput=False)
    betal_d = nc.declare_dram_parameter("betal", [HL, M], bf16, isOutput=False)
    cd_d = nc.declare_dram_parameter("cd", [N, HL + 1], f32, isOutput=False)
    tri_d = nc.declare_dram_parameter("tri", [4 * P, MT], f32, isOutput=False)
    outp_d = nc.declare_dram_parameter("outp", [E, M], f32, isOutput=True)

    with tile.TileContext(nc) as tc:
        with (
            tc.tile_pool(name="const", bufs=1) as const,
            tc.tile_pool(name="vstream", bufs=2) as vstream,
            tc.tile_pool(name="bcast", bufs=2) as bcast,
            tc.tile_pool(name="work", bufs=3) as work,
            tc.tile_pool(name="et_pool", bufs=17) as et_pool,
            tc.tile_pool(name="small", bufs=4) as small,
            tc.tile_pool(name="opool", bufs=3) as opool,
            tc.tile_pool(name="ps_v", bufs=2, space="PSUM") as ps_v,
            tc.tile_pool(name="ps_pool", bufs=2, space="PSUM") as ps_pool,
            tc.tile_pool(name="ps_tp", bufs=2, space="PSUM") as ps_tp,
            tc.tile_pool(name="ps_o", bufs=2, space="PSUM") as ps_o,
        ):
            # ---- resident constants ----
            wvlt_sb = const.tile([P, E // P, KL], f32)
            nc.sync.dma_start(wvlt_sb[:], wvlt_d.rearrange("(ek p) d -> p ek d", p=P))
            wolt_sb = const.tile([P, KL // P, E], f32)
            nc.sync.dma_start(wolt_sb[:], wolt_d.rearrange("(kb p) o -> p kb o", p=P))
            cd_sb = const.tile([P, NK, HL + 1], f32)
            nc.sync.dma_start(cd_sb[:], cd_d.rearrange("(k p) f -> p k f", p=P))
            tri_sb = const.tile([P, 4, MT], f32)
            nc.sync.dma_start(tri_sb[:], tri_d.rearrange("(pos p) m -> p pos m", p=P))
            ident = const.tile([P, P], f32)
            make_identity(nc, ident[:])

            # v_sb[:, k, h*65 : h*65+64] = v for head h, chunk k; col h*65+64 = 1.0
            v_sb = const.tile([P, NK, HL * (DH + 1)], bf16)
            nc.vector.memset(v_sb[:], 1.0)

            # ---- stage 1: v projection ----
            for q in range(4):  # n-quarters of 512
                vt_sb = vstream.tile([P, E // P, MT], f32)
                nc.sync.dma_start(
                    vt_sb[:],
                    vt_d[:, q * MT:(q + 1) * MT].rearrange("(ek p) n -> p ek n", p=P),
                )
                for nk_r in range(4):
                    k = q * 4 + nk_r
                    vps = ps_v.tile([P, KL], f32)
                    for ek in range(E // P):
                        nc.tensor.matmul(
                            vps[:],
                            vt_sb[:, ek, nk_r * P:(nk_r + 1) * P],
                            wvlt_sb[:, ek, :],
                            start=(ek == 0),
                            stop=(ek == E // P - 1),
                        )
                    nc.any.tensor_copy(
                        out=v_sb[:, k].rearrange("p (h x) -> p h x", x=DH + 1)[:, :, 0:DH],
                        in_=vps.rearrange("p (h x) -> p h x", x=DH),
                    )

            # ---- stage 2: scores / softmax / attention / output ----
            for t in range(T):
                qsb = bcast.tile([P, HL, MT], f32, tag="qsb")
                nc.sync.dma_start(
                    qsb[:],
                    qsl_d[None, :, t * MT:(t + 1) * MT].to_broadcast([P, HL, MT]),
                )
                bb = bcast.tile([P, HL, MT], bf16, tag="bb")
                nc.sync.dma_start(
                    bb[:],
                    betal_d[None, :, t * MT:(t + 1) * MT].to_broadcast([P, HL, MT]),
                )

                pooln = work.tile([P, 4, KL], f32, tag="pooln")
                for h in range(HL):
                    pool_ps = ps_pool.tile([P, 4, DH + 1], f32)
                    ets = {}
                    for k in range(4 * t, NK):
                        sc = work.tile([P, MT], f32, tag="sc")
                        nc.vector.scalar_tensor_tensor(
                            out=sc[:],
                            in0=qsb[:, h],
                            scalar=cd_sb[:, k, h:h + 1],
                            in1=bb[:, h],
                            op0=mybir.AluOpType.mult,
                            op1=mybir.AluOpType.add,
                        )
                        pos = k - 4 * t
                        if pos < 4:
                            nc.vector.tensor_add(
                                out=sc[:], in0=sc[:], in1=tri_sb[:, pos]
                            )
                        et = et_pool.tile([P, MT], bf16, tag="et")
                        nc.scalar.activation(
                            et[:], sc[:], mybir.ActivationFunctionType.Exp,
                            bias=cd_sb[:, k, HL:HL + 1],
                        )
                        ets[k] = et
                    for ms in range(4):
                        for k in range(4 * t + ms, NK):
                            nc.tensor.matmul(
                                pool_ps[:, ms],
                                ets[k][:, ms * P:(ms + 1) * P],
                                v_sb[:, k, h * (DH + 1):(h + 1) * (DH + 1)],
                                start=(k == 4 * t + ms),
                                stop=(k == NK - 1),
                            )
                    rec = small.tile([P, 4], f32, tag="rec")
                    nc.vector.reciprocal(rec[:], pool_ps[:, :, DH])
                    for ms in range(4):
                        nc.vector.tensor_scalar_mul(
                            pooln[:, ms, h * DH:(h + 1) * DH],
                            pool_ps[:, ms, 0:DH],
                            rec[:, ms:ms + 1],
                        )

                # transpose pooled [m, k] -> pT [k, m] and out-projection
                pT = work.tile([P, KL // P, MT], f32, tag="pT")
                for ms in range(4):
                    for kb in range(KL // P):
                        tps = ps_tp.tile([P, P], f32)
                        nc.tensor.transpose(
                            tps[:], pooln[:, ms, kb * P:(kb + 1) * P], ident[:]
                        )
                        nc.any.tensor_copy(
                            out=pT[:, kb, ms * P:(ms + 1) * P], in_=tps[:]
                        )
                for ob in range(E // P):
                    ops = ps_o.tile([P, MT], f32)
                    for kb in range(KL // P):
                        nc.tensor.matmul(
                            ops[:],
                            wolt_sb[:, kb, ob * P:(ob + 1) * P],
                            pT[:, kb],
                            start=(kb == 0),
                            stop=(kb == KL // P - 1),
                        )
                    osb = opool.tile([P, MT], f32, tag="osb")
                    nc.any.tensor_copy(out=osb[:], in_=ops[:])
                    nc.sync.dma_start(
                        outp_d[ob * P:(ob + 1) * P, t * MT:(t + 1) * MT], osb[:]
                    )

    nc.compile()
    _CACHE["nc"] = nc
    return nc


def _host_prep(queries, keys, values, Wq, bq, Wk, bk, Wv, bv, Wo, bo, in_mask):
    """Host-side prep. Returns (in_maps, fixup, extras)."""
    qs = np.einsum("mbe,he->mbh", queries, Wq.reshape(H, DH, E).sum(1),
                   dtype=np.float32) + bq.reshape(H, DH).sum(1)
    ks = np.einsum("nbe,he->nbh", keys, Wk.reshape(H, DH, E).sum(1),
                   dtype=np.float32) + bk.reshape(H, DH).sum(1)

    mask3 = in_mask[:, :, None]
    cp = np.where(mask3, 0.0, ks).astype(np.float32)          # [n, b, H]
    d = np.where(in_mask, NEG, 0.0).astype(np.float32)        # [n, b]

    cmax = np.where(mask3, -np.inf, ks)
    cmax = np.maximum.accumulate(cmax[::-1], axis=0)[::-1]    # suffix max, n>=m
    cmin = np.where(mask3, np.inf, ks)
    cmin = np.minimum.accumulate(cmin[::-1], axis=0)[::-1]
    nonempty = np.maximum.accumulate((~in_mask)[::-1], axis=0)[::-1]  # [n, b]

    with np.errstate(invalid="ignore"):
        A = np.where(qs >= 0, qs * cmax, qs * cmin)           # [m, b, H]
    A = np.where(nonempty[:, :, None], A, -np.inf)
    fixup_rows = np.any(~(A > -990.0), axis=2)                # [m, b] (nan-safe)
    beta = np.where(np.isfinite(A), -A, 1e4)
    beta = np.where(np.any(~(A > -990.0), axis=2)[:, :, None], -1e4, beta)
    beta = beta.astype(np.float32)

    in_maps = []
    vt_by_b = [np.ascontiguousarray(values[:, bi, :].T) for bi in range(B)]
    tri = np.zeros((4 * P, MT), np.float32)
    for pos in range(4):
        nr = np.arange(P)[:, None] + 128 * pos
        mr = np.arange(MT)[None, :]
        tri[pos * P:(pos + 1) * P] = np.where(nr < mr, -4000.0, 0.0)

    for c in range(NCORES):
        bi, hg = c // 4, c % 4
        lh = slice(hg * HL, (hg + 1) * HL)
        ds = slice(hg * KL, (hg + 1) * KL)
        in_maps.append({
            "vt": vt_by_b[bi],
            "wvlt": np.ascontiguousarray(Wv[ds, :].T),
            "wolt": np.ascontiguousarray(Wo[:, ds].T),
            "qsl": np.ascontiguousarray(qs[:, bi, lh].T),
            "betal": np.ascontiguousarray(beta[:, bi, lh].T).astype(ml_dtypes.bfloat16),
            "cd": np.ascontiguousarray(
                np.concatenate([cp[:, bi, lh], d[:, bi:bi + 1]], axis=1)),
            "tri": tri,
        })
    return in_maps, fixup_rows, (qs, ks)


def _fixup_row(out, m, bi, qs, ks, values, Wv, bv, Wo, bo, in_mask):
    """Exact numpy recompute of one output row (degenerate / extreme rows)."""
    pot = qs[m, bi, :][None, :] * ks[:, bi, :]                # [n, H]
    pot = np.where(in_mask[:, bi][:, None], NEG, pot)
    causal = np.arange(N) < m                                 # mask n < m
    pot = np.where(causal[:, None], NEG, pot)
    pot = pot - pot.max(axis=0, keepdims=True)
    w = np.exp(pot)
    w = w / w.sum(axis=0, keepdims=True)                      # [n, H]
    v = (values[:, bi, :] @ Wv.T + bv).reshape(N, H, DH)
    pooled = np.einsum("nh,nhd->hd", w, v).reshape(E)
    out[m, bi, :] = pooled @ Wo.T + bo


def kernel(queries, keys, values, Wq, bq, Wk, bk, Wv, bv, Wo, bo, in_mask,
           _trace=False):
    args = (queries, keys, values, Wq, bq, Wk, bk, Wv, bv, Wo, bo)
    args = tuple(np.asarray(a, np.float32) for a in args)
    in_mask = np.asarray(in_mask, bool)
    (queries, keys, values, Wq, bq, Wk, bk, Wv, bv, Wo, bo) = args

    nc = _build_program()
    in_maps, fixup_rows, (qs, ks) = _host_prep(
        queries, keys, values, Wq, bq, Wk, bk, Wv, bv, Wo, bo, in_mask)

    res = run_bass_kernel_spmd(nc, in_maps, list(range(NCORES)), trace=_trace)
    results = res.results

    out = np.zeros((M, B, E), np.float32)
    for c in range(NCORES):
        bi = c // 4
        out[:, bi, :] += np.asarray(results[c]["outp"], np.float32).T
    out += (bo + bv @ Wo.T)[None, None, :]

    for m, bi in zip(*np.nonzero(fixup_rows)):
        _fixup_row(out, m, bi, qs, ks, values, Wv, bv, Wo, bo, in_mask)

    if _trace:
        return out, res
    return out
